# revision 1
# baseline (speedup 1.0000x reference)
"""Int8 AG-GEMM (x @ weight.T with per-row/per-col dequant + bias) on 8 TRN2
NeuronCores.

Strategy: data-parallel over M (rows of x). Core c owns rows
[c*512, (c+1)*512). All inputs are fed fully prepared from the host in the
exact SBUF tile layout, so every DMA source is contiguous per partition
(descriptor generation is then negligible):
  - xt   [XC, 128, K/(128*XC), M_C] int8 : transposed x shard, chunked over K
  - wt   [N/NB, WQ, 128, K/(128*WQ), NB] int8 : transposed weight, tiled
         (replicated to every core)
  - isr  [128, M_C] f32 : input_scale shard replicated across partitions
  - wsr  [128, N/128] f32 : weight_scale, partition-major
  - br   [128, N/128] f32 : bias, partition-major
Each core computes outT = [N, M_C] bf16 (the transposed output shard):
  psum[n-tile 128, M_C] = sum_k wt_tile[k, n].T @ xt_tile[k]   (fp32, exact)
  out = (psum * isr) * ws[n][:,1] + bias[n][:,1] -> bf16  (two DVE ops)
The host transposes each core's outT back and stitches the full [M, N].

The int8 GEMM is exact: int8 values are exact in bf16, products are exact in
the PE's fp32 accumulator, and partial sums stay far below 2^24.

DMA queues: the int8 x/weight streams ride the gpsimd SWDGE queue, which
casts int8->bf16 in flight (halves HBM traffic); block-0 weight quarters
are interleaved with the x chunks in first-use order so the PE starts
~17us in. Scales ride the scalar HWDGE queue; output stores ride sync.
"""

import numpy as np

M_FULL, K_FULL, N_FULL = 4096, 8192, 8192
N_CORES = 8
N_XCHUNK = 8    # x load split (SWDGE granules)
N_WQUART = 4    # weight block k-split (SWDGE granules)
N_PER_BLK = 256


def build_nc(K, N, M_C, n_per_blk=N_PER_BLK):
    """Build the SPMD kernel graph for per-core problem [K, N] x [K, M_C]."""
    import concourse.mybir as mybir
    import concourse.tile as tile
    from concourse import bacc

    bf16 = mybir.dt.bfloat16
    f32 = mybir.dt.float32

    kt = K // 128          # k-tiles
    nt = N // 128          # n-tiles (output partition tiles)
    nblk = N // n_per_blk  # weight streaming blocks
    jt = n_per_blk // 128  # n-tiles per block
    xc_n = min(N_XCHUNK, kt)
    kc = kt // xc_n        # k-tiles per x chunk
    wq_n = min(N_WQUART, kt)
    kq = kt // wq_n        # k-tiles per weight quarter

    i8 = mybir.dt.int8

    nc = bacc.Bacc("TRN2", target_bir_lowering=False, debug=False,
                   num_devices=N_CORES)
    xt = nc.dram_tensor("xt", [xc_n, 128, kc, M_C], i8, kind="ExternalInput")
    wt = nc.dram_tensor("wt", [nblk, wq_n, 128, kq, n_per_blk], i8,
                        kind="ExternalInput")
    # duplicate copies of the k=0 tiles: tiny first DMAs so the first
    # matmul fires before the bulk granules finish streaming
    xk0 = nc.dram_tensor("xk0", [128, M_C], i8, kind="ExternalInput")
    wk0 = nc.dram_tensor("wk0", [128, n_per_blk], i8, kind="ExternalInput")
    isr = nc.dram_tensor("isr", [128, M_C], f32, kind="ExternalInput")
    wsr = nc.dram_tensor("wsr", [128, nt], f32, kind="ExternalInput")
    br = nc.dram_tensor("br", [128, nt], f32, kind="ExternalInput")
    outt = nc.dram_tensor("outt", [N, M_C], bf16, kind="ExternalOutput")

    with tile.TileContext(nc) as tc:
        with (
            tc.tile_pool(name="const", bufs=1) as cpool,
            tc.tile_pool(name="wstream", bufs=3 * wq_n) as wpool,
            tc.tile_pool(name="psum", bufs=4, space="PSUM") as ppool,
            tc.tile_pool(name="t1", bufs=4) as t1pool,
            tc.tile_pool(name="osb", bufs=4) as opool,
        ):
            xch = [cpool.tile([128, kc, M_C], bf16, tag=f"xsb{c}",
                              name=f"xsb{c}")
                   for c in range(xc_n)]

            def dma_x(c):
                nc.gpsimd.dma_start(xch[c][:], xt.ap()[c])

            # Interleave x-chunk loads with block-0 weight quarters on the
            # SWDGE queue so the first psum group's deps land first: the
            # group's k-range for weight quarter q needs x chunks 2q, 2q+1.
            x_per_q = max(1, xc_n // wq_n)
            xk0_sb = cpool.tile([128, M_C], bf16)
            nc.gpsimd.dma_start(xk0_sb[:], xk0.ap())
            wk0_sb = cpool.tile([128, n_per_blk], bf16)
            nc.gpsimd.dma_start(wk0_sb[:], wk0.ap())
            dma_x(0)
            x_issued = 1
            isr_sb = cpool.tile([128, M_C], f32)
            nc.scalar.dma_start(isr_sb[:], isr.ap())
            ws_sb = cpool.tile([128, nt], f32)
            nc.scalar.dma_start(ws_sb[:], wsr.ap())
            b_sb = cpool.tile([128, nt], f32)
            nc.scalar.dma_start(b_sb[:], br.ap())

            for s in range(nblk):
                wqs = []
                for q in range(wq_n):
                    wq = wpool.tile([128, kq, n_per_blk], bf16, tag="wq")
                    nc.gpsimd.dma_start(wq[:], wt.ap()[s, q])
                    wqs.append(wq)
                    if s == 0:
                        for _ in range(x_per_q):
                            if x_issued < xc_n:
                                dma_x(x_issued)
                                x_issued += 1
                while x_issued < xc_n:
                    dma_x(x_issued)
                    x_issued += 1
                for j in range(jt):
                    n = s * jt + j
                    ps = ppool.tile([128, M_C], f32)
                    for k in range(kt):
                        if s == 0 and k == 0:
                            wsrc = wk0_sb[:, j * 128:(j + 1) * 128]
                            xsrc = xk0_sb[:]
                        else:
                            wsrc = wqs[k // kq][:, k % kq,
                                               j * 128:(j + 1) * 128]
                            xsrc = xch[k // kc][:, k % kc, :]
                        nc.tensor.matmul(
                            ps[:], wsrc, xsrc,
                            start=(k == 0),
                            stop=(k == kt - 1),
                        )
                    t1 = t1pool.tile([128, M_C], f32)
                    nc.vector.tensor_tensor(
                        t1[:], ps[:], isr_sb[:], mybir.AluOpType.mult
                    )
                    ob = opool.tile([128, M_C], bf16)
                    nc.vector.tensor_scalar(
                        ob[:], t1[:],
                        ws_sb[:, n:n + 1], b_sb[:, n:n + 1],
                        mybir.AluOpType.mult, mybir.AluOpType.add,
                    )
                    nc.sync.dma_start(outt.ap()[n * 128:(n + 1) * 128, :], ob[:])

    nc.compile()
    return nc


def prep_in_maps(x, weight, bias, input_scale, weight_scale, n_cores=N_CORES,
                 n_per_blk=N_PER_BLK):
    """Host-side shard + SBUF-layout prep. Returns (in_maps, M_C)."""
    import ml_dtypes

    bf16 = ml_dtypes.bfloat16
    M, K = x.shape
    N = weight.shape[0]
    M_C = M // n_cores
    kt = K // 128
    xc_n = min(N_XCHUNK, kt)
    kc = kt // xc_n
    wq_n = min(N_WQUART, kt)
    kq = kt // wq_n
    nblk = N // n_per_blk

    xt_full = np.ascontiguousarray(x.T).astype(np.int8)  # [K, M]
    wt = np.ascontiguousarray(weight.T).astype(np.int8)  # [K, N]
    # [K, N] -> [nblk, wq_n, 128, kq, n_per_blk];  K = wq_n*kq*128
    wt_t = np.ascontiguousarray(
        wt.reshape(wq_n, kq, 128, nblk, n_per_blk).transpose(3, 0, 2, 1, 4))
    wsr = np.ascontiguousarray(
        weight_scale.astype(np.float32).reshape(N // 128, 128).T)
    br = np.ascontiguousarray(bias.astype(np.float32).reshape(N // 128, 128).T)

    in_maps = []
    for c in range(n_cores):
        sl = slice(c * M_C, (c + 1) * M_C)
        # [K, M_C] -> [xc_n, 128, kc, M_C]
        xt_c = np.ascontiguousarray(
            xt_full[:, sl].reshape(xc_n, kc, 128, M_C).transpose(0, 2, 1, 3))
        in_maps.append({
            "xt": xt_c,
            "wt": wt_t,
            "xk0": np.ascontiguousarray(xt_c[0, :, 0, :]),
            "wk0": np.ascontiguousarray(wt_t[0, 0, :, 0, :]),
            "isr": np.ascontiguousarray(
                np.broadcast_to(input_scale[sl].astype(np.float32)[None, :],
                                (128, M_C))),
            "wsr": wsr,
            "br": br,
        })
    return in_maps, M_C


def run(x, weight, bias, input_scale, weight_scale, trace=False):
    """Run the SPMD kernel; returns (out [M, N] bf16, BassKernelResults)."""
    from concourse.bass_utils import run_bass_kernel_spmd

    M, K = x.shape
    N = weight.shape[0]
    in_maps, M_C = prep_in_maps(x, weight, bias, input_scale, weight_scale)
    nc = build_nc(K, N, M_C)
    res = run_bass_kernel_spmd(nc, in_maps, list(range(N_CORES)), trace=trace)

    import ml_dtypes
    out = np.empty((M, N), dtype=ml_dtypes.bfloat16)
    for c in range(N_CORES):
        out[c * M_C:(c + 1) * M_C, :] = res.results[c]["outt"].T
    return out, res


def kernel(x, weight, bias, input_scale, weight_scale):
    x, weight, bias, input_scale, weight_scale = (
        np.asarray(a) for a in (x, weight, bias, input_scale, weight_scale))
    out, _ = run(x, weight, bias, input_scale, weight_scale, trace=False)
    return out



# revision 2
# speedup vs baseline: 1.1203x; 1.1203x over previous
"""Int8 AG-GEMM (x @ weight.T with per-row/per-col dequant + bias) on 8 TRN2
NeuronCores.

Strategy: data-parallel over M (rows of x). Core c owns rows
[c*512, (c+1)*512). The PE does the whole GEMM in bf16 (int8 values are
exact in bf16; products are exact in the fp32 PSUM accumulator), 4096
matmuls of [128k x 128n] x [128k x 512m] per core — that is the hardware
floor of 1 int8 product per PE cell per cycle.

v2 layout (vs the v1 baseline):
  - x is pre-scaled by input_scale on the host and shipped as bf16
    ([128, 64, 512] k-tile-major). This folds the per-row dequant into the
    operand (rel. rounding error ~2^-9, far inside the 2e-2 gate) so the
    per-n-tile post-processing is a single DVE tensor_scalar
    (ps * weight_scale[n] + bias[n] -> bf16), and x needs no in-flight
    cast, so it can ride the hardware DGE queues (sync engine) which start
    ~4us before the gpsimd software-DGE ring.
  - Startup is need-ordered across all three DMA queues: sync streams the
    x chunks in k order (first chunk only 2 k-tiles so the first matmul
    fires ~9us in), scalar streams scales/bias + the first weight block
    (host-precast bf16, k-progressive granules [2,6,8,16,32]), gpsimd
    streams weight blocks 1..31 as int8->bf16 half-block granules. The
    aggregate DMA rate (~420 GB/s, shared across queues) paces the first
    block; the PE is never idle-waiting on a coarse granule.
  - Weight blocks 1..31 ride the SWDGE queue as [128, 32, 256] halves
    (8 KiB/partition packets), 6-buffer pool = 3 blocks of lookahead.

Each core computes outT = [N, M_C] bf16 (transposed output shard); the host
transposes each core's outT back and stitches the full [M, N].
"""

import numpy as np

M_FULL, K_FULL, N_FULL = 4096, 8192, 8192
N_CORES = 8
N_PER_BLK = 256
KT = K_FULL // 128              # 64 k-tiles
NBLK = N_FULL // N_PER_BLK      # 32 weight blocks
X_CHUNK_KT = (2, 6, 8, 8, 8, 8, 8, 8, 8)     # k-tiles per x chunk (sum 64)
W0_CHUNK_KT = (2, 6, 8, 16, 32)              # k-tiles per block-0 granule


def _starts(sizes):
    out, a = [], 0
    for s in sizes:
        out.append(a)
        a += s
    return out


def build_nc(M_C):
    """Build the SPMD kernel graph for one core's [K, N] x [K, M_C]."""
    import concourse.mybir as mybir
    import concourse.tile as tile
    from concourse import bacc

    bf16 = mybir.dt.bfloat16
    f32 = mybir.dt.float32
    i8 = mybir.dt.int8

    xst = _starts(X_CHUNK_KT)
    w0st = _starts(W0_CHUNK_KT)

    nc = bacc.Bacc("TRN2", target_bir_lowering=False, debug=False,
                   num_devices=N_CORES)
    xt = nc.dram_tensor("xt", [128, KT, M_C], bf16, kind="ExternalInput")
    w0 = nc.dram_tensor("w0", [128, KT, N_PER_BLK], bf16,
                        kind="ExternalInput")
    wh = nc.dram_tensor("wh", [NBLK - 1, 2, 128, KT // 2, N_PER_BLK], i8,
                        kind="ExternalInput")
    wsr = nc.dram_tensor("wsr", [128, N_FULL // 128], f32,
                         kind="ExternalInput")
    br = nc.dram_tensor("br", [128, N_FULL // 128], f32,
                        kind="ExternalInput")
    outt = nc.dram_tensor("outt", [N_FULL, M_C], bf16, kind="ExternalOutput")

    with tile.TileContext(nc) as tc:
        with (
            tc.tile_pool(name="const", bufs=1) as cpool,
            tc.tile_pool(name="wstream", bufs=6) as wpool,
            tc.tile_pool(name="psum", bufs=4, space="PSUM") as ppool,
            tc.tile_pool(name="osb", bufs=4) as opool,
        ):
            # scalar queue: scales/bias, then block-0 weight granules
            ws_sb = cpool.tile([128, N_FULL // 128], f32)
            nc.scalar.dma_start(ws_sb[:], wsr.ap())
            b_sb = cpool.tile([128, N_FULL // 128], f32)
            nc.scalar.dma_start(b_sb[:], br.ap())
            w0g = []
            for gi, (a, sz) in enumerate(zip(w0st, W0_CHUNK_KT)):
                t = cpool.tile([128, sz, N_PER_BLK], bf16, name=f"w0g{gi}")
                nc.scalar.dma_start(t[:], w0.ap()[:, a:a + sz, :])
                w0g.append(t)
            # sync queue: x chunks in k order
            xch = []
            for ci, (a, sz) in enumerate(zip(xst, X_CHUNK_KT)):
                t = cpool.tile([128, sz, M_C], bf16, name=f"xch{ci}")
                nc.sync.dma_start(t[:], xt.ap()[:, a:a + sz, :])
                xch.append(t)

            def xsrc(k):
                for c in range(len(xst) - 1, -1, -1):
                    if k >= xst[c]:
                        return xch[c][:, k - xst[c], :]
                raise AssertionError

            def w0src(k, j):
                for g in range(len(w0st) - 1, -1, -1):
                    if k >= w0st[g]:
                        return w0g[g][:, k - w0st[g],
                                      j * 128:(j + 1) * 128]
                raise AssertionError

            # gpsimd SWDGE: weight blocks 1..31, half-block granules
            halves = {}
            for s in range(1, NBLK):
                for h in range(2):
                    t = wpool.tile([128, KT // 2, N_PER_BLK], bf16, tag="wh")
                    nc.gpsimd.dma_start(t[:], wh.ap()[s - 1, h])
                    halves[(s, h)] = t

            for n in range(2 * NBLK):
                s, j = n // 2, n % 2
                ps = ppool.tile([128, M_C], f32)
                for k in range(KT):
                    if s == 0:
                        wsrc = w0src(k, j)
                    else:
                        t = halves[(s, k // (KT // 2))]
                        wsrc = t[:, k % (KT // 2), j * 128:(j + 1) * 128]
                    nc.tensor.matmul(
                        ps[:], wsrc, xsrc(k),
                        start=(k == 0),
                        stop=(k == KT - 1),
                    )
                ob = opool.tile([128, M_C], bf16)
                nc.vector.tensor_scalar(
                    ob[:], ps[:],
                    ws_sb[:, n:n + 1], b_sb[:, n:n + 1],
                    mybir.AluOpType.mult, mybir.AluOpType.add,
                )
                nc.sync.dma_start(outt.ap()[n * 128:(n + 1) * 128, :], ob[:])

    nc.compile()
    return nc


def prep_in_maps(x, weight, bias, input_scale, weight_scale,
                 n_cores=N_CORES):
    """Host-side shard + SBUF-layout prep. Returns (in_maps, M_C)."""
    import ml_dtypes

    bf16 = ml_dtypes.bfloat16
    M, K = x.shape
    N = weight.shape[0]
    M_C = M // n_cores
    kt = K // 128

    # x pre-scaled by input_scale, bf16, transposed, k-tile-major
    xs = (x.astype(np.float32)
          * input_scale.astype(np.float32)[:, None]).astype(bf16)  # [M, K]
    # [K, M] -> [kt, 128, M] -> per-core [128, kt, M_C]
    xt3 = xs.T.reshape(kt, 128, M)

    wt = np.ascontiguousarray(weight.T).astype(np.int8)  # [K, N]
    # block 0 as bf16 [128, kt, 256]
    w0 = np.ascontiguousarray(
        wt[:, :N_PER_BLK].astype(bf16).reshape(kt, 128, N_PER_BLK)
        .transpose(1, 0, 2))
    # blocks 1.. as int8 halves [nblk-1, 2, 128, kt/2, 256]
    nblk = N // N_PER_BLK
    whx = np.ascontiguousarray(
        wt[:, N_PER_BLK:]                      # [K, N-256]
        .reshape(2, kt // 2, 128, nblk - 1, N_PER_BLK)
        .transpose(3, 0, 2, 1, 4))
    wsr = np.ascontiguousarray(
        weight_scale.astype(np.float32).reshape(N // 128, 128).T)
    br = np.ascontiguousarray(
        bias.astype(np.float32).reshape(N // 128, 128).T)

    in_maps = []
    for c in range(n_cores):
        sl = slice(c * M_C, (c + 1) * M_C)
        xt_c = np.ascontiguousarray(xt3[:, :, sl].transpose(1, 0, 2))
        in_maps.append({
            "xt": xt_c,
            "w0": w0,
            "wh": whx,
            "wsr": wsr,
            "br": br,
        })
    return in_maps, M_C


def run(x, weight, bias, input_scale, weight_scale, trace=False):
    """Run the SPMD kernel; returns (out [M, N] bf16, BassKernelResults)."""
    from concourse.bass_utils import run_bass_kernel_spmd

    M, K = x.shape
    N = weight.shape[0]
    in_maps, M_C = prep_in_maps(x, weight, bias, input_scale, weight_scale)
    nc = build_nc(M_C)
    res = run_bass_kernel_spmd(nc, in_maps, list(range(N_CORES)), trace=trace)

    import ml_dtypes
    out = np.empty((M, N), dtype=ml_dtypes.bfloat16)
    for c in range(N_CORES):
        out[c * M_C:(c + 1) * M_C, :] = res.results[c]["outt"].T
    return out, res


def kernel(x, weight, bias, input_scale, weight_scale):
    x, weight, bias, input_scale, weight_scale = (
        np.asarray(a) for a in (x, weight, bias, input_scale, weight_scale))
    out, _ = run(x, weight, bias, input_scale, weight_scale, trace=False)
    return out


# revision 3
# speedup vs baseline: 1.1943x; 1.0661x over previous
"""Int8 AG-GEMM (x @ weight.T with per-row/per-col dequant + bias) on 8 TRN2
NeuronCores.

Strategy: data-parallel over M (rows of x). Core c owns rows
[c*512, (c+1)*512). The PE does the whole GEMM in bf16 (int8 values are
exact in bf16; products are exact in the fp32 PSUM accumulator): 4096
matmuls of [128k x 128n] x [128k x 512m] per core — the hardware floor of
one int8 product per PE cell per cycle (~216 ns per matmul warm).

All startup-critical traffic rides the single gpsimd SWDGE queue in exact
first-use order (cross-queue DMA arbitration is unfair under contention;
the aggregate rate ~420 GB/s is shared, so parallel queues add nothing):
x chunks (int8 -> bf16 in-flight cast, k-progressive sizes so the first
matmul fires ~11 us in) interleaved with block-0 weight granules
(k-progressive), then weight blocks 1..31 as [128, 16, 256] quarters.
Scales/bias ride the scalar HWDGE queue; output stores ride sync.

Per n-tile post-processing: in-place psum *= input_scale (DVE
tensor_tensor, [128, M_C] broadcast), then psum * weight_scale[n] +
bias[n] -> bf16 (DVE tensor_scalar). No intermediate SBUF tile.

Each core computes outT = [N, M_C] bf16 (transposed output shard); the
host transposes each core's outT back and stitches the full [M, N].
"""

import numpy as np

M_FULL, K_FULL, N_FULL = 4096, 8192, 8192
N_CORES = 8
N_PER_BLK = 256
KT = K_FULL // 128              # 64 k-tiles
NBLK = N_FULL // N_PER_BLK      # 32 weight blocks
KQ = 16                         # k-tiles per streamed weight quarter
X_CHUNK_KT = (2, 6, 8, 8, 8, 8, 8, 8, 8)     # k-tiles per x chunk (sum 64)
W0_CHUNK_KT = (2, 6, 8, 16, 16, 16)          # k-tiles per block-0 granule
# interleaved q0 issue order for the startup stream: (kind, index)
STARTUP_ORDER = (
    ("x", 0), ("w", 0), ("x", 1), ("w", 1), ("x", 2), ("w", 2),
    ("x", 3), ("x", 4), ("w", 3), ("x", 5), ("w", 4), ("x", 6),
    ("x", 7), ("w", 5), ("x", 8),
)


def _starts(sizes):
    out, a = [], 0
    for s in sizes:
        out.append(a)
        a += s
    return out


def build_nc(M_C):
    """Build the SPMD kernel graph for one core's [K, N] x [K, M_C]."""
    import concourse.mybir as mybir
    import concourse.tile as tile
    from concourse import bacc

    bf16 = mybir.dt.bfloat16
    f32 = mybir.dt.float32
    i8 = mybir.dt.int8

    xst = _starts(X_CHUNK_KT)
    w0st = _starts(W0_CHUNK_KT)

    nc = bacc.Bacc("TRN2", target_bir_lowering=False, debug=False,
                   num_devices=N_CORES)
    xt = nc.dram_tensor("xt", [128, KT, M_C], i8, kind="ExternalInput")
    w0 = nc.dram_tensor("w0", [128, KT, N_PER_BLK], i8, kind="ExternalInput")
    wq = nc.dram_tensor("wq", [NBLK - 1, KT // KQ, 128, KQ, N_PER_BLK], i8,
                        kind="ExternalInput")
    isr = nc.dram_tensor("isr", [128, M_C], f32, kind="ExternalInput")
    wsr = nc.dram_tensor("wsr", [128, N_FULL // 128], f32,
                         kind="ExternalInput")
    br = nc.dram_tensor("br", [128, N_FULL // 128], f32,
                        kind="ExternalInput")
    outt = nc.dram_tensor("outt", [N_FULL, M_C], bf16, kind="ExternalOutput")

    with tile.TileContext(nc) as tc:
        with (
            tc.tile_pool(name="const", bufs=1) as cpool,
            tc.tile_pool(name="wstream", bufs=10) as wpool,
            tc.tile_pool(name="psum", bufs=4, space="PSUM") as ppool,
            tc.tile_pool(name="osb", bufs=4) as opool,
        ):
            # scalar HWDGE queue: scales + bias (small, off the hot queue)
            isr_sb = cpool.tile([128, M_C], f32)
            nc.scalar.dma_start(isr_sb[:], isr.ap())
            ws_sb = cpool.tile([128, N_FULL // 128], f32)
            nc.scalar.dma_start(ws_sb[:], wsr.ap())
            b_sb = cpool.tile([128, N_FULL // 128], f32)
            nc.scalar.dma_start(b_sb[:], br.ap())

            # gpsimd SWDGE queue, exact first-use order:
            # x chunks and block-0 granules interleaved by k-range
            xch = [cpool.tile([128, sz, M_C], bf16, name=f"xch{ci}")
                   for ci, sz in enumerate(X_CHUNK_KT)]
            w0g = [cpool.tile([128, sz, N_PER_BLK], bf16, name=f"w0g{gi}")
                   for gi, sz in enumerate(W0_CHUNK_KT)]
            for kind, i in STARTUP_ORDER:
                if kind == "x":
                    a, sz = xst[i], X_CHUNK_KT[i]
                    nc.gpsimd.dma_start(xch[i][:], xt.ap()[:, a:a + sz, :])
                else:
                    a, sz = w0st[i], W0_CHUNK_KT[i]
                    nc.gpsimd.dma_start(w0g[i][:], w0.ap()[:, a:a + sz, :])

            def xsrc(k):
                for c in range(len(xst) - 1, -1, -1):
                    if k >= xst[c]:
                        return xch[c][:, k - xst[c], :]
                raise AssertionError

            def w0src(k, j):
                for g in range(len(w0st) - 1, -1, -1):
                    if k >= w0st[g]:
                        return w0g[g][:, k - w0st[g], j * 128:(j + 1) * 128]
                raise AssertionError

            # then weight blocks 1..31 as quarters on the same queue
            quarters = {}
            for s in range(1, NBLK):
                for q in range(KT // KQ):
                    t = wpool.tile([128, KQ, N_PER_BLK], bf16, tag="wq")
                    nc.gpsimd.dma_start(t[:], wq.ap()[s - 1, q])
                    quarters[(s, q)] = t

            for n in range(2 * NBLK):
                s, j = n // 2, n % 2
                ps = ppool.tile([128, M_C], f32)
                for k in range(KT):
                    if s == 0:
                        wsrc = w0src(k, j)
                    else:
                        t = quarters[(s, k // KQ)]
                        wsrc = t[:, k % KQ, j * 128:(j + 1) * 128]
                    nc.tensor.matmul(
                        ps[:], wsrc, xsrc(k),
                        start=(k == 0),
                        stop=(k == KT - 1),
                    )
                # dequant in place, then scale+bias -> bf16
                nc.vector.tensor_tensor(
                    ps[:], ps[:], isr_sb[:], mybir.AluOpType.mult
                )
                ob = opool.tile([128, M_C], bf16)
                nc.vector.tensor_scalar(
                    ob[:], ps[:],
                    ws_sb[:, n:n + 1], b_sb[:, n:n + 1],
                    mybir.AluOpType.mult, mybir.AluOpType.add,
                )
                nc.sync.dma_start(outt.ap()[n * 128:(n + 1) * 128, :], ob[:])

    nc.compile()
    return nc


def prep_in_maps(x, weight, bias, input_scale, weight_scale,
                 n_cores=N_CORES):
    """Host-side shard + SBUF-layout prep. Returns (in_maps, M_C)."""
    M, K = x.shape
    N = weight.shape[0]
    M_C = M // n_cores
    kt = K // 128

    # [K, M] -> [kt, 128, M]
    xt3 = np.ascontiguousarray(x.T).astype(np.int8).reshape(kt, 128, M)

    wt = np.ascontiguousarray(weight.T).astype(np.int8)  # [K, N]
    w0 = np.ascontiguousarray(
        wt[:, :N_PER_BLK].reshape(kt, 128, N_PER_BLK).transpose(1, 0, 2))
    nblk = N // N_PER_BLK
    wqx = np.ascontiguousarray(
        wt[:, N_PER_BLK:]
        .reshape(kt // KQ, KQ, 128, nblk - 1, N_PER_BLK)
        .transpose(3, 0, 2, 1, 4))
    wsr = np.ascontiguousarray(
        weight_scale.astype(np.float32).reshape(N // 128, 128).T)
    br = np.ascontiguousarray(
        bias.astype(np.float32).reshape(N // 128, 128).T)

    in_maps = []
    for c in range(n_cores):
        sl = slice(c * M_C, (c + 1) * M_C)
        xt_c = np.ascontiguousarray(xt3[:, :, sl].transpose(1, 0, 2))
        in_maps.append({
            "xt": xt_c,
            "w0": w0,
            "wq": wqx,
            "isr": np.ascontiguousarray(
                np.broadcast_to(input_scale[sl].astype(np.float32)[None, :],
                                (128, M_C))),
            "wsr": wsr,
            "br": br,
        })
    return in_maps, M_C


def run(x, weight, bias, input_scale, weight_scale, trace=False):
    """Run the SPMD kernel; returns (out [M, N] bf16, BassKernelResults)."""
    from concourse.bass_utils import run_bass_kernel_spmd

    M, K = x.shape
    N = weight.shape[0]
    in_maps, M_C = prep_in_maps(x, weight, bias, input_scale, weight_scale)
    nc = build_nc(M_C)
    res = run_bass_kernel_spmd(nc, in_maps, list(range(N_CORES)), trace=trace)

    import ml_dtypes
    out = np.empty((M, N), dtype=ml_dtypes.bfloat16)
    for c in range(N_CORES):
        out[c * M_C:(c + 1) * M_C, :] = res.results[c]["outt"].T
    return out, res


def kernel(x, weight, bias, input_scale, weight_scale):
    x, weight, bias, input_scale, weight_scale = (
        np.asarray(a) for a in (x, weight, bias, input_scale, weight_scale))
    out, _ = run(x, weight, bias, input_scale, weight_scale, trace=False)
    return out


# revision 9
# speedup vs baseline: 1.1952x; 1.0008x over previous
"""Int8 AG-GEMM (x @ weight.T with per-row/per-col dequant + bias) on 8 TRN2
NeuronCores.

Strategy: data-parallel over M (rows of x). Core c owns rows
[c*512, (c+1)*512). The PE does the whole GEMM in bf16 (int8 values are
exact in bf16; products are exact in the fp32 PSUM accumulator): 4096
matmuls of [128k x 128n] x [128k x 512m] per core — the hardware floor of
one int8 product per PE cell per cycle (~216 ns per matmul warm).

All startup-critical traffic rides the single gpsimd SWDGE queue in exact
first-use order (cross-queue DMA arbitration is unfair under contention;
the aggregate rate ~420 GB/s is shared, so parallel queues add nothing):
x chunks (int8 -> bf16 in-flight cast, k-progressive sizes so the first
matmul fires ~11 us in) interleaved with block-0 weight granules
(k-progressive), then weight blocks 1..31 as [128, 16, 256] quarters.
Scales/bias ride the scalar HWDGE queue; output stores ride sync.

Per n-tile post-processing: in-place psum *= input_scale (DVE
tensor_tensor, [128, M_C] broadcast), then psum * weight_scale[n] +
bias[n] -> bf16 (DVE tensor_scalar). No intermediate SBUF tile.

Each core computes outT = [N, M_C] bf16 (transposed output shard); the
host transposes each core's outT back and stitches the full [M, N].
"""

import numpy as np

M_FULL, K_FULL, N_FULL = 4096, 8192, 8192
N_CORES = 8
N_PER_BLK = 256
KT = K_FULL // 128              # 64 k-tiles
NBLK = N_FULL // N_PER_BLK      # 32 weight blocks
KQ = 16                         # k-tiles per streamed weight quarter
X_CHUNK_KT = (2, 6, 8, 8, 8, 8, 8, 8, 8)     # k-tiles per x chunk (sum 64)
W0_CHUNK_KT = (2, 6, 8, 16, 8, 8, 8, 8)      # k-tiles per block-0 granule
# chunk 0 of each (k-tiles 0-1) is host-precast bf16 and rides the sync /
# scalar HWDGE queues, which start ~2 us before the gpsimd SWDGE ring.
# The rest streams on gpsimd (q0) in this interleaved first-use order:
STARTUP_ORDER = (
    ("x", 1), ("w", 1), ("x", 2), ("w", 2), ("x", 3), ("x", 4),
    ("w", 3), ("x", 5), ("w", 4), ("x", 6), ("w", 5), ("x", 7),
    ("w", 6), ("x", 8), ("w", 7),
)


def _starts(sizes):
    out, a = [], 0
    for s in sizes:
        out.append(a)
        a += s
    return out


def build_nc(M_C):
    """Build the SPMD kernel graph for one core's [K, N] x [K, M_C]."""
    import concourse.mybir as mybir
    import concourse.tile as tile
    from concourse import bacc

    bf16 = mybir.dt.bfloat16
    f32 = mybir.dt.float32
    i8 = mybir.dt.int8

    xst = _starts(X_CHUNK_KT)
    w0st = _starts(W0_CHUNK_KT)

    nc = bacc.Bacc("TRN2", target_bir_lowering=False, debug=False,
                   num_devices=N_CORES)
    xt = nc.dram_tensor("xt", [128, KT, M_C], i8, kind="ExternalInput")
    x0b = nc.dram_tensor("x0b", [128, X_CHUNK_KT[0], M_C], bf16,
                         kind="ExternalInput")
    w0b = nc.dram_tensor("w0b", [128, W0_CHUNK_KT[0], N_PER_BLK], bf16,
                         kind="ExternalInput")
    w0 = nc.dram_tensor("w0", [128, KT, N_PER_BLK], i8, kind="ExternalInput")
    wq = nc.dram_tensor("wq", [NBLK - 1, KT // KQ, 128, KQ, N_PER_BLK], i8,
                        kind="ExternalInput")
    isr = nc.dram_tensor("isr", [128, M_C], f32, kind="ExternalInput")
    wsr = nc.dram_tensor("wsr", [128, N_FULL // 128], f32,
                         kind="ExternalInput")
    br = nc.dram_tensor("br", [128, N_FULL // 128], f32,
                        kind="ExternalInput")
    outt = nc.dram_tensor("outt", [N_FULL, M_C], bf16, kind="ExternalOutput")

    with tile.TileContext(nc) as tc:
        with (
            tc.tile_pool(name="const", bufs=1) as cpool,
            tc.tile_pool(name="wstream", bufs=8) as wpool,
            tc.tile_pool(name="psum", bufs=4, space="PSUM") as ppool,
            tc.tile_pool(name="osb", bufs=4) as opool,
        ):
            xch = [cpool.tile([128, sz, M_C], bf16, name=f"xch{ci}")
                   for ci, sz in enumerate(X_CHUNK_KT)]
            w0g = [cpool.tile([128, sz, N_PER_BLK], bf16, name=f"w0g{gi}")
                   for gi, sz in enumerate(W0_CHUNK_KT)]
            # first k-tiles (host-precast bf16) on the early HWDGE queues
            nc.sync.dma_start(xch[0][:], x0b.ap())
            nc.scalar.dma_start(w0g[0][:], w0b.ap())
            # scalar HWDGE queue: scales + bias (small, off the hot queue)
            isr_sb = cpool.tile([128, M_C], f32)
            nc.scalar.dma_start(isr_sb[:], isr.ap())
            ws_sb = cpool.tile([128, N_FULL // 128], f32)
            nc.scalar.dma_start(ws_sb[:], wsr.ap())
            b_sb = cpool.tile([128, N_FULL // 128], f32)
            nc.scalar.dma_start(b_sb[:], br.ap())

            # gpsimd SWDGE queue, exact first-use order:
            # x chunks and block-0 granules interleaved by k-range
            for kind, i in STARTUP_ORDER:
                if kind == "x":
                    a, sz = xst[i], X_CHUNK_KT[i]
                    nc.gpsimd.dma_start(xch[i][:], xt.ap()[:, a:a + sz, :])
                else:
                    a, sz = w0st[i], W0_CHUNK_KT[i]
                    nc.gpsimd.dma_start(w0g[i][:], w0.ap()[:, a:a + sz, :])

            def xsrc(k):
                for c in range(len(xst) - 1, -1, -1):
                    if k >= xst[c]:
                        return xch[c][:, k - xst[c], :]
                raise AssertionError

            def w0src(k, j):
                for g in range(len(w0st) - 1, -1, -1):
                    if k >= w0st[g]:
                        return w0g[g][:, k - w0st[g], j * 128:(j + 1) * 128]
                raise AssertionError

            # then weight blocks 1..31 as quarters on the same queue
            quarters = {}
            for s in range(1, NBLK):
                for q in range(KT // KQ):
                    t = wpool.tile([128, KQ, N_PER_BLK], bf16, tag="wq")
                    nc.gpsimd.dma_start(t[:], wq.ap()[s - 1, q])
                    quarters[(s, q)] = t

            for n in range(2 * NBLK):
                s, j = n // 2, n % 2
                ps = ppool.tile([128, M_C], f32)
                for k in range(KT):
                    if s == 0:
                        wsrc = w0src(k, j)
                    else:
                        t = quarters[(s, k // KQ)]
                        wsrc = t[:, k % KQ, j * 128:(j + 1) * 128]
                    nc.tensor.matmul(
                        ps[:], wsrc, xsrc(k),
                        start=(k == 0),
                        stop=(k == KT - 1),
                    )
                # dequant in place, then scale+bias -> bf16
                nc.vector.tensor_tensor(
                    ps[:], ps[:], isr_sb[:], mybir.AluOpType.mult
                )
                ob = opool.tile([128, M_C], bf16)
                nc.vector.tensor_scalar(
                    ob[:], ps[:],
                    ws_sb[:, n:n + 1], b_sb[:, n:n + 1],
                    mybir.AluOpType.mult, mybir.AluOpType.add,
                )
                nc.sync.dma_start(outt.ap()[n * 128:(n + 1) * 128, :], ob[:])

    nc.compile()
    return nc


def prep_in_maps(x, weight, bias, input_scale, weight_scale,
                 n_cores=N_CORES):
    """Host-side shard + SBUF-layout prep. Returns (in_maps, M_C)."""
    M, K = x.shape
    N = weight.shape[0]
    M_C = M // n_cores
    kt = K // 128

    import ml_dtypes
    bf16 = ml_dtypes.bfloat16

    # [K, M] -> [kt, 128, M]
    xt3 = np.ascontiguousarray(x.T).astype(np.int8).reshape(kt, 128, M)

    wt = np.ascontiguousarray(weight.T).astype(np.int8)  # [K, N]
    w0 = np.ascontiguousarray(
        wt[:, :N_PER_BLK].reshape(kt, 128, N_PER_BLK).transpose(1, 0, 2))
    w0b = np.ascontiguousarray(w0[:, :W0_CHUNK_KT[0], :]).astype(bf16)
    nblk = N // N_PER_BLK
    wqx = np.ascontiguousarray(
        wt[:, N_PER_BLK:]
        .reshape(kt // KQ, KQ, 128, nblk - 1, N_PER_BLK)
        .transpose(3, 0, 2, 1, 4))
    wsr = np.ascontiguousarray(
        weight_scale.astype(np.float32).reshape(N // 128, 128).T)
    br = np.ascontiguousarray(
        bias.astype(np.float32).reshape(N // 128, 128).T)

    in_maps = []
    for c in range(n_cores):
        sl = slice(c * M_C, (c + 1) * M_C)
        xt_c = np.ascontiguousarray(xt3[:, :, sl].transpose(1, 0, 2))
        in_maps.append({
            "xt": xt_c,
            "x0b": np.ascontiguousarray(
                xt_c[:, :X_CHUNK_KT[0], :]).astype(bf16),
            "w0b": w0b,
            "w0": w0,
            "wq": wqx,
            "isr": np.ascontiguousarray(
                np.broadcast_to(input_scale[sl].astype(np.float32)[None, :],
                                (128, M_C))),
            "wsr": wsr,
            "br": br,
        })
    return in_maps, M_C


def run(x, weight, bias, input_scale, weight_scale, trace=False):
    """Run the SPMD kernel; returns (out [M, N] bf16, BassKernelResults)."""
    from concourse.bass_utils import run_bass_kernel_spmd

    M, K = x.shape
    N = weight.shape[0]
    in_maps, M_C = prep_in_maps(x, weight, bias, input_scale, weight_scale)
    nc = build_nc(M_C)
    res = run_bass_kernel_spmd(nc, in_maps, list(range(N_CORES)), trace=trace)

    import ml_dtypes
    out = np.empty((M, N), dtype=ml_dtypes.bfloat16)
    for c in range(N_CORES):
        out[c * M_C:(c + 1) * M_C, :] = res.results[c]["outt"].T
    return out, res


def kernel(x, weight, bias, input_scale, weight_scale):
    x, weight, bias, input_scale, weight_scale = (
        np.asarray(a) for a in (x, weight, bias, input_scale, weight_scale))
    out, _ = run(x, weight, bias, input_scale, weight_scale, trace=False)
    return out


# revision 11
# speedup vs baseline: 1.1987x; 1.0029x over previous
"""Int8 AG-GEMM (x @ weight.T with per-row/per-col dequant + bias) on 8 TRN2
NeuronCores.

Strategy: data-parallel over M (rows of x). Core c owns rows
[c*512, (c+1)*512). The PE does the whole GEMM in bf16 (int8 values are
exact in bf16; products are exact in the fp32 PSUM accumulator): 4096
matmuls of [128k x 128n] x [128k x 512m] per core — the hardware floor of
one int8 product per PE cell per cycle (~216 ns per matmul warm).

Startup is the only schedule-sensitive part: all queues share ~420 GB/s,
so the first output block's data (8 MB bf16 of x + weights) is DMA-bound
against its own compute. To absorb that, phase A runs the first FOUR
n-tiles (cols 0..511) k-synchronously on four PSUM banks: each fresh
k-tile of x + weights enables 4 matmuls instead of 1, giving ~55 us of PE
runway against ~16 MB of startup traffic — the PE never starves and the
HAM clock gate never re-throttles. Weights for phase A are shipped
k-major ([128, kt, 512] granules); x chunks are k-progressive. The first
two k-tiles of both are host-precast bf16 and ride the sync/scalar HWDGE
queues, which start ~2 us before the gpsimd SWDGE ring; everything else
streams int8 -> bf16 on gpsimd in exact first-use order. Phase B (blocks
2..31, n-block-major [128, 16, 256] quarters) only needs the steady-state
~148 GB/s weight stream.

Per n-tile post-processing: in-place psum *= input_scale (DVE
tensor_tensor, [128, M_C] broadcast), then psum * weight_scale[n] +
bias[n] -> bf16 (DVE tensor_scalar). Output stores ride sync.

Each core computes outT = [N, M_C] bf16 (transposed output shard); the
host transposes each core's outT back and stitches the full [M, N].
"""

import numpy as np

M_FULL, K_FULL, N_FULL = 4096, 8192, 8192
N_CORES = 8
N_PER_BLK = 256
KT = K_FULL // 128              # 64 k-tiles
NBLK = N_FULL // N_PER_BLK      # 32 weight blocks
KQ = 16                         # k-tiles per streamed weight quarter
NA = 4                          # phase-A n-tiles (cols 0 .. NA*128-1)
X_CHUNK_KT = (2, 6, 8, 8, 8, 8, 8, 8, 8)     # k-tiles per x chunk (sum 64)
WA_CHUNK_KT = (2, 6, 8, 8, 8, 8, 8, 8, 8)    # k-tiles per phase-A w granule
# gpsimd issue order for the startup stream (chunk 0 of each rides the
# earlier-starting HWDGE queues as host-precast bf16)
STARTUP_ORDER = (
    ("x", 1), ("w", 1), ("x", 2), ("w", 2), ("x", 3), ("w", 3),
    ("x", 4), ("w", 4), ("x", 5), ("w", 5), ("x", 6), ("w", 6),
    ("x", 7), ("w", 7), ("x", 8), ("w", 8),
)


def _starts(sizes):
    out, a = [], 0
    for s in sizes:
        out.append(a)
        a += s
    return out


def build_nc(M_C):
    """Build the SPMD kernel graph for one core's [K, N] x [K, M_C]."""
    import concourse.mybir as mybir
    import concourse.tile as tile
    from concourse import bacc

    bf16 = mybir.dt.bfloat16
    f32 = mybir.dt.float32
    i8 = mybir.dt.int8

    xst = _starts(X_CHUNK_KT)
    wast = _starts(WA_CHUNK_KT)

    nc = bacc.Bacc("TRN2", target_bir_lowering=False, debug=False,
                   num_devices=N_CORES)
    xt = nc.dram_tensor("xt", [128, KT, M_C], i8, kind="ExternalInput")
    x0b = nc.dram_tensor("x0b", [128, X_CHUNK_KT[0], M_C], bf16,
                         kind="ExternalInput")
    wa = nc.dram_tensor("wa", [128, KT, NA * 128], i8, kind="ExternalInput")
    wa0b = nc.dram_tensor("wa0b", [128, WA_CHUNK_KT[0], NA * 128], bf16,
                          kind="ExternalInput")
    wq = nc.dram_tensor("wq", [NBLK - NA // 2, KT // KQ, 128, KQ, N_PER_BLK],
                        i8, kind="ExternalInput")
    isr = nc.dram_tensor("isr", [128, M_C], f32, kind="ExternalInput")
    wsr = nc.dram_tensor("wsr", [128, N_FULL // 128], f32,
                         kind="ExternalInput")
    br = nc.dram_tensor("br", [128, N_FULL // 128], f32,
                        kind="ExternalInput")
    outt = nc.dram_tensor("outt", [N_FULL, M_C], bf16, kind="ExternalOutput")

    with tile.TileContext(nc) as tc:
        with (
            tc.tile_pool(name="const", bufs=1) as cpool,
            tc.tile_pool(name="wstream", bufs=8) as wpool,
            tc.tile_pool(name="psum", bufs=6, space="PSUM") as ppool,
            tc.tile_pool(name="osb", bufs=4) as opool,
        ):
            xch = [cpool.tile([128, sz, M_C], bf16, name=f"xch{ci}")
                   for ci, sz in enumerate(X_CHUNK_KT)]
            wag = [cpool.tile([128, sz, NA * 128], bf16, name=f"wag{gi}")
                   for gi, sz in enumerate(WA_CHUNK_KT)]
            # first k-tiles (host-precast bf16) on the early HWDGE queues
            nc.sync.dma_start(xch[0][:], x0b.ap())
            nc.scalar.dma_start(wag[0][:], wa0b.ap())
            # scalar HWDGE queue: scales + bias (small)
            isr_sb = cpool.tile([128, M_C], f32)
            nc.scalar.dma_start(isr_sb[:], isr.ap())
            ws_sb = cpool.tile([128, N_FULL // 128], f32)
            nc.scalar.dma_start(ws_sb[:], wsr.ap())
            b_sb = cpool.tile([128, N_FULL // 128], f32)
            nc.scalar.dma_start(b_sb[:], br.ap())

            # gpsimd SWDGE queue, exact first-use order
            for kind, i in STARTUP_ORDER:
                if kind == "x":
                    a, sz = xst[i], X_CHUNK_KT[i]
                    nc.gpsimd.dma_start(xch[i][:], xt.ap()[:, a:a + sz, :])
                else:
                    a, sz = wast[i], WA_CHUNK_KT[i]
                    nc.gpsimd.dma_start(wag[i][:], wa.ap()[:, a:a + sz, :])

            def xsrc(k):
                for c in range(len(xst) - 1, -1, -1):
                    if k >= xst[c]:
                        return xch[c][:, k - xst[c], :]
                raise AssertionError

            def wasrc(k, g):
                for c in range(len(wast) - 1, -1, -1):
                    if k >= wast[c]:
                        return wag[c][:, k - wast[c], g * 128:(g + 1) * 128]
                raise AssertionError

            # then weight blocks NA/2 .. NBLK-1 as quarters
            quarters = {}
            for s in range(NA // 2, NBLK):
                for q in range(KT // KQ):
                    t = wpool.tile([128, KQ, N_PER_BLK], bf16, tag="wq")
                    nc.gpsimd.dma_start(t[:], wq.ap()[s - NA // 2, q])
                    quarters[(s, q)] = t

            def postproc(ps, n):
                nc.vector.tensor_tensor(
                    ps[:], ps[:], isr_sb[:], mybir.AluOpType.mult
                )
                ob = opool.tile([128, M_C], bf16)
                nc.vector.tensor_scalar(
                    ob[:], ps[:],
                    ws_sb[:, n:n + 1], b_sb[:, n:n + 1],
                    mybir.AluOpType.mult, mybir.AluOpType.add,
                )
                nc.sync.dma_start(outt.ap()[n * 128:(n + 1) * 128, :], ob[:])

            # phase A: n-tiles 0..NA-1 k-synchronously on NA psum banks
            psa = [ppool.tile([128, M_C], f32, tag="ps", name=f"psa{g}")
                   for g in range(NA)]
            for k in range(KT):
                xs = xsrc(k)
                for g in range(NA):
                    nc.tensor.matmul(
                        psa[g][:], wasrc(k, g), xs,
                        start=(k == 0),
                        stop=(k == KT - 1),
                    )
            for g in range(NA):
                postproc(psa[g], g)

            # phase B: n-tiles NA..63, block-major
            for n in range(NA, 2 * NBLK):
                s, j = n // 2, n % 2
                ps = ppool.tile([128, M_C], f32, tag="ps")
                for k in range(KT):
                    t = quarters[(s, k // KQ)]
                    wsrc = t[:, k % KQ, j * 128:(j + 1) * 128]
                    nc.tensor.matmul(
                        ps[:], wsrc, xsrc(k),
                        start=(k == 0),
                        stop=(k == KT - 1),
                    )
                postproc(ps, n)

    nc.compile()
    return nc


def prep_in_maps(x, weight, bias, input_scale, weight_scale,
                 n_cores=N_CORES):
    """Host-side shard + SBUF-layout prep. Returns (in_maps, M_C)."""
    import ml_dtypes
    bf16 = ml_dtypes.bfloat16

    M, K = x.shape
    N = weight.shape[0]
    M_C = M // n_cores
    kt = K // 128

    # [K, M] -> [kt, 128, M]
    xt3 = np.ascontiguousarray(x.T).astype(np.int8).reshape(kt, 128, M)

    wt = np.ascontiguousarray(weight.T).astype(np.int8)  # [K, N]
    # phase-A strip: cols 0 .. NA*128-1, k-tile-major [128, kt, NA*128]
    wa = np.ascontiguousarray(
        wt[:, :NA * 128].reshape(kt, 128, NA * 128).transpose(1, 0, 2))
    wa0b = np.ascontiguousarray(wa[:, :WA_CHUNK_KT[0], :]).astype(bf16)
    # blocks NA/2 .. : [nblk - NA/2, kt/KQ, 128, KQ, 256]
    nblk = N // N_PER_BLK
    wqx = np.ascontiguousarray(
        wt[:, NA * 128:]
        .reshape(kt // KQ, KQ, 128, nblk - NA // 2, N_PER_BLK)
        .transpose(3, 0, 2, 1, 4))
    wsr = np.ascontiguousarray(
        weight_scale.astype(np.float32).reshape(N // 128, 128).T)
    br = np.ascontiguousarray(
        bias.astype(np.float32).reshape(N // 128, 128).T)

    in_maps = []
    for c in range(n_cores):
        sl = slice(c * M_C, (c + 1) * M_C)
        xt_c = np.ascontiguousarray(xt3[:, :, sl].transpose(1, 0, 2))
        in_maps.append({
            "xt": xt_c,
            "x0b": np.ascontiguousarray(
                xt_c[:, :X_CHUNK_KT[0], :]).astype(bf16),
            "wa": wa,
            "wa0b": wa0b,
            "wq": wqx,
            "isr": np.ascontiguousarray(
                np.broadcast_to(input_scale[sl].astype(np.float32)[None, :],
                                (128, M_C))),
            "wsr": wsr,
            "br": br,
        })
    return in_maps, M_C


def run(x, weight, bias, input_scale, weight_scale, trace=False):
    """Run the SPMD kernel; returns (out [M, N] bf16, BassKernelResults)."""
    from concourse.bass_utils import run_bass_kernel_spmd

    M, K = x.shape
    N = weight.shape[0]
    in_maps, M_C = prep_in_maps(x, weight, bias, input_scale, weight_scale)
    nc = build_nc(M_C)
    res = run_bass_kernel_spmd(nc, in_maps, list(range(N_CORES)), trace=trace)

    import ml_dtypes
    out = np.empty((M, N), dtype=ml_dtypes.bfloat16)
    for c in range(N_CORES):
        out[c * M_C:(c + 1) * M_C, :] = res.results[c]["outt"].T
    return out, res


def kernel(x, weight, bias, input_scale, weight_scale):
    x, weight, bias, input_scale, weight_scale = (
        np.asarray(a) for a in (x, weight, bias, input_scale, weight_scale))
    out, _ = run(x, weight, bias, input_scale, weight_scale, trace=False)
    return out


# revision 13
# speedup vs baseline: 1.1990x; 1.0003x over previous
"""Int8 AG-GEMM (x @ weight.T with per-row/per-col dequant + bias) on 8 TRN2
NeuronCores.

Strategy: data-parallel over M (rows of x). Core c owns rows
[c*512, (c+1)*512). The PE does the whole GEMM in bf16 (int8 values are
exact in bf16; products are exact in the fp32 PSUM accumulator): 4096
matmuls of [128k x 128n] x [128k x 512m] per core — the hardware floor of
one int8 product per PE cell per cycle (~216 ns per matmul warm).

Startup is the only schedule-sensitive part: all queues share ~420 GB/s,
so the first output block's data (8 MB bf16 of x + weights) is DMA-bound
against its own compute. To absorb that, phase A runs the first FOUR
n-tiles (cols 0..511) k-synchronously on four PSUM banks: each fresh
k-tile of x + weights enables 4 matmuls instead of 1, giving ~55 us of PE
runway against ~16 MB of startup traffic — the PE never starves and the
HAM clock gate never re-throttles. Weights for phase A are shipped
k-major ([128, kt, 512] granules); x chunks are k-progressive. The first
two k-tiles of both are host-precast bf16 and ride the sync/scalar HWDGE
queues, which start ~2 us before the gpsimd SWDGE ring; everything else
streams int8 -> bf16 on gpsimd in exact first-use order. Phase B (blocks
2..31, n-block-major [128, 16, 256] quarters) only needs the steady-state
~148 GB/s weight stream.

Per n-tile post-processing: in-place psum *= input_scale (DVE
tensor_tensor, [128, M_C] broadcast), then psum * weight_scale[n] +
bias[n] -> bf16 (DVE tensor_scalar). Output stores ride sync.

Each core computes outT = [N, M_C] bf16 (transposed output shard); the
host transposes each core's outT back and stitches the full [M, N].
"""

import numpy as np

M_FULL, K_FULL, N_FULL = 4096, 8192, 8192
N_CORES = 8
N_PER_BLK = 256
KT = K_FULL // 128              # 64 k-tiles
NBLK = N_FULL // N_PER_BLK      # 32 weight blocks
KQ = 16                         # k-tiles per streamed weight quarter
NA = 4                          # phase-A n-tiles (cols 0 .. NA*128-1)
X_CHUNK_KT = (1, 7, 8, 16, 16, 16)           # k-tiles per x chunk (sum 64)
WA_CHUNK_KT = (1, 7, 8, 16, 16, 16)          # k-tiles per phase-A w granule
N_WARMUP_MM = 36                             # HAM pre-warm dummy matmuls
# gpsimd issue order for the startup stream (chunk 0 of each rides the
# earlier-starting HWDGE queues as host-precast bf16); w leads x since
# phase A consumes 4 matmuls per k-tile
STARTUP_ORDER = (
    ("w", 1), ("x", 1), ("w", 2), ("x", 2), ("w", 3), ("x", 3),
    ("w", 4), ("x", 4), ("w", 5), ("x", 5),
)


def _starts(sizes):
    out, a = [], 0
    for s in sizes:
        out.append(a)
        a += s
    return out


def build_nc(M_C):
    """Build the SPMD kernel graph for one core's [K, N] x [K, M_C]."""
    import concourse.mybir as mybir
    import concourse.tile as tile
    from concourse import bacc

    bf16 = mybir.dt.bfloat16
    f32 = mybir.dt.float32
    i8 = mybir.dt.int8

    xst = _starts(X_CHUNK_KT)
    wast = _starts(WA_CHUNK_KT)

    nc = bacc.Bacc("TRN2", target_bir_lowering=False, debug=False,
                   num_devices=N_CORES)
    xt = nc.dram_tensor("xt", [128, KT, M_C], i8, kind="ExternalInput")
    x0b = nc.dram_tensor("x0b", [128, X_CHUNK_KT[0], M_C], bf16,
                         kind="ExternalInput")
    wa = nc.dram_tensor("wa", [128, KT, NA * 128], i8, kind="ExternalInput")
    wa0b = nc.dram_tensor("wa0b", [128, WA_CHUNK_KT[0], NA * 128], bf16,
                          kind="ExternalInput")
    wq = nc.dram_tensor("wq", [NBLK - NA // 2, KT // KQ, 128, KQ, N_PER_BLK],
                        i8, kind="ExternalInput")
    isr = nc.dram_tensor("isr", [128, M_C], f32, kind="ExternalInput")
    wsr = nc.dram_tensor("wsr", [128, N_FULL // 128], f32,
                         kind="ExternalInput")
    br = nc.dram_tensor("br", [128, N_FULL // 128], f32,
                        kind="ExternalInput")
    outt = nc.dram_tensor("outt", [N_FULL, M_C], bf16, kind="ExternalOutput")

    with tile.TileContext(nc) as tc:
        with (
            tc.tile_pool(name="const", bufs=1) as cpool,
            tc.tile_pool(name="wstream", bufs=8) as wpool,
            tc.tile_pool(name="psum", bufs=6, space="PSUM") as ppool,
            tc.tile_pool(name="psdmy", bufs=1, space="PSUM") as dpool,
            tc.tile_pool(name="osb", bufs=4) as opool,
        ):
            # HAM pre-warm: keep the PE busy from the end of the preamble
            # so the clock gate is at 8/8 before the first real matmul.
            dmy = cpool.tile([128, 128], bf16, name="dmy")
            nc.gpsimd.memset(dmy[:], 0)
            dps = dpool.tile([128, 128], f32)
            for _ in range(N_WARMUP_MM):
                nc.tensor.matmul(dps[:], dmy[:], dmy[:],
                                 start=True, stop=True)

            xch = [cpool.tile([128, sz, M_C], bf16, name=f"xch{ci}")
                   for ci, sz in enumerate(X_CHUNK_KT)]
            wag = [cpool.tile([128, sz, NA * 128], bf16, name=f"wag{gi}")
                   for gi, sz in enumerate(WA_CHUNK_KT)]
            # first k-tiles (host-precast bf16) on the early HWDGE queues
            nc.sync.dma_start(xch[0][:], x0b.ap())
            nc.scalar.dma_start(wag[0][:], wa0b.ap())
            # scalar HWDGE queue: scales + bias (small)
            isr_sb = cpool.tile([128, M_C], f32)
            nc.scalar.dma_start(isr_sb[:], isr.ap())
            ws_sb = cpool.tile([128, N_FULL // 128], f32)
            nc.scalar.dma_start(ws_sb[:], wsr.ap())
            b_sb = cpool.tile([128, N_FULL // 128], f32)
            nc.scalar.dma_start(b_sb[:], br.ap())

            # gpsimd SWDGE queue, exact first-use order
            for kind, i in STARTUP_ORDER:
                if kind == "x":
                    a, sz = xst[i], X_CHUNK_KT[i]
                    nc.gpsimd.dma_start(xch[i][:], xt.ap()[:, a:a + sz, :])
                else:
                    a, sz = wast[i], WA_CHUNK_KT[i]
                    nc.gpsimd.dma_start(wag[i][:], wa.ap()[:, a:a + sz, :])

            def xsrc(k):
                for c in range(len(xst) - 1, -1, -1):
                    if k >= xst[c]:
                        return xch[c][:, k - xst[c], :]
                raise AssertionError

            def wasrc(k, g):
                for c in range(len(wast) - 1, -1, -1):
                    if k >= wast[c]:
                        return wag[c][:, k - wast[c], g * 128:(g + 1) * 128]
                raise AssertionError

            # then weight blocks NA/2 .. NBLK-1 as quarters
            quarters = {}
            for s in range(NA // 2, NBLK):
                for q in range(KT // KQ):
                    t = wpool.tile([128, KQ, N_PER_BLK], bf16, tag="wq")
                    nc.gpsimd.dma_start(t[:], wq.ap()[s - NA // 2, q])
                    quarters[(s, q)] = t

            def postproc(ps, n):
                nc.vector.tensor_tensor(
                    ps[:], ps[:], isr_sb[:], mybir.AluOpType.mult
                )
                ob = opool.tile([128, M_C], bf16)
                nc.vector.tensor_scalar(
                    ob[:], ps[:],
                    ws_sb[:, n:n + 1], b_sb[:, n:n + 1],
                    mybir.AluOpType.mult, mybir.AluOpType.add,
                )
                nc.sync.dma_start(outt.ap()[n * 128:(n + 1) * 128, :], ob[:])

            # phase A: n-tiles 0..NA-1 k-synchronously on NA psum banks
            psa = [ppool.tile([128, M_C], f32, tag="ps", name=f"psa{g}")
                   for g in range(NA)]
            for k in range(KT):
                xs = xsrc(k)
                for g in range(NA):
                    nc.tensor.matmul(
                        psa[g][:], wasrc(k, g), xs,
                        start=(k == 0),
                        stop=(k == KT - 1),
                    )
            for g in range(NA):
                postproc(psa[g], g)

            # phase B: n-tiles NA..63, block-major
            for n in range(NA, 2 * NBLK):
                s, j = n // 2, n % 2
                ps = ppool.tile([128, M_C], f32, tag="ps")
                for k in range(KT):
                    t = quarters[(s, k // KQ)]
                    wsrc = t[:, k % KQ, j * 128:(j + 1) * 128]
                    nc.tensor.matmul(
                        ps[:], wsrc, xsrc(k),
                        start=(k == 0),
                        stop=(k == KT - 1),
                    )
                postproc(ps, n)

    nc.compile()
    return nc


def prep_in_maps(x, weight, bias, input_scale, weight_scale,
                 n_cores=N_CORES):
    """Host-side shard + SBUF-layout prep. Returns (in_maps, M_C)."""
    import ml_dtypes
    bf16 = ml_dtypes.bfloat16

    M, K = x.shape
    N = weight.shape[0]
    M_C = M // n_cores
    kt = K // 128

    # [K, M] -> [kt, 128, M]
    xt3 = np.ascontiguousarray(x.T).astype(np.int8).reshape(kt, 128, M)

    wt = np.ascontiguousarray(weight.T).astype(np.int8)  # [K, N]
    # phase-A strip: cols 0 .. NA*128-1, k-tile-major [128, kt, NA*128]
    wa = np.ascontiguousarray(
        wt[:, :NA * 128].reshape(kt, 128, NA * 128).transpose(1, 0, 2))
    wa0b = np.ascontiguousarray(wa[:, :WA_CHUNK_KT[0], :]).astype(bf16)
    # blocks NA/2 .. : [nblk - NA/2, kt/KQ, 128, KQ, 256]
    nblk = N // N_PER_BLK
    wqx = np.ascontiguousarray(
        wt[:, NA * 128:]
        .reshape(kt // KQ, KQ, 128, nblk - NA // 2, N_PER_BLK)
        .transpose(3, 0, 2, 1, 4))
    wsr = np.ascontiguousarray(
        weight_scale.astype(np.float32).reshape(N // 128, 128).T)
    br = np.ascontiguousarray(
        bias.astype(np.float32).reshape(N // 128, 128).T)

    in_maps = []
    for c in range(n_cores):
        sl = slice(c * M_C, (c + 1) * M_C)
        xt_c = np.ascontiguousarray(xt3[:, :, sl].transpose(1, 0, 2))
        in_maps.append({
            "xt": xt_c,
            "x0b": np.ascontiguousarray(
                xt_c[:, :X_CHUNK_KT[0], :]).astype(bf16),
            "wa": wa,
            "wa0b": wa0b,
            "wq": wqx,
            "isr": np.ascontiguousarray(
                np.broadcast_to(input_scale[sl].astype(np.float32)[None, :],
                                (128, M_C))),
            "wsr": wsr,
            "br": br,
        })
    return in_maps, M_C


def run(x, weight, bias, input_scale, weight_scale, trace=False):
    """Run the SPMD kernel; returns (out [M, N] bf16, BassKernelResults)."""
    from concourse.bass_utils import run_bass_kernel_spmd

    M, K = x.shape
    N = weight.shape[0]
    in_maps, M_C = prep_in_maps(x, weight, bias, input_scale, weight_scale)
    nc = build_nc(M_C)
    res = run_bass_kernel_spmd(nc, in_maps, list(range(N_CORES)), trace=trace)

    import ml_dtypes
    out = np.empty((M, N), dtype=ml_dtypes.bfloat16)
    for c in range(N_CORES):
        out[c * M_C:(c + 1) * M_C, :] = res.results[c]["outt"].T
    return out, res


def kernel(x, weight, bias, input_scale, weight_scale):
    x, weight, bias, input_scale, weight_scale = (
        np.asarray(a) for a in (x, weight, bias, input_scale, weight_scale))
    out, _ = run(x, weight, bias, input_scale, weight_scale, trace=False)
    return out
